# revision 7
# baseline (speedup 1.0000x reference)
"""Multi-head self-attention (B=2, S=2048, D=1024, H=16, causal) on 8 TRN2 cores.

Sharding: core c handles batch b=c//4 and head-group g=c%4 (4 heads each).
Host pre-transposes/pre-tiles everything into bf16 so on-chip there are no
transposes and every DMA is one per-partition-contiguous transfer:
  xt   [4][128, 8*512]  xt[sb][p, ko*512+s] = x[b].T[ko*128+p, sb*512+s]
  wq/wk/wv [128, 8, 256] w[p,ko,m] = W.T[ko*128+p, g*256+m]
  wo   [128, 2, 1024]    wo[p,co,n] = Wo[:, g*256+co*128+p].T row
  tri  [128, 128]        tri[j,i] = (j <= i)  (diagonal-block causal mask)
Host sums the 4 per-group bf16 partial outputs per batch at the end (fp32).

Schedule (all matmuls bf16 into fp32 PSUM, built for zero PE idle so the
HAM clock gate stays at 8/8):
  - few, large DMAs: the Sync sequencer's per-DMA descriptor generation was
    ~8us of dead time with 40 small DMAs; now it is 10 DMAs total.
  - projections are interleaved with attention blocks (P(sb0) V(0:4) A0
    P(sb1) V(4:8) A1 ...) so the ACT engine's exp work (~0.9ns/elem, the
    secondary bottleneck) spreads over the whole kernel instead of
    rate-limiting the attention phase.
  - v tiles [128, 4(head), 128]: cols 0:64 hold v, cols 64:128 hold 1.0, so
    the PV matmul accumulates the numerator in PSUM rows 0:64 AND the
    softmax denominator replicated across rows 64:128; normalization is
    recip+mul on DVE only (the denominator is staged through a plain
    tensor_copy first: custom-DVE ops misread partition-offset PSUM
    operands on HW).
  - scoresT[j,i] layout: softmax needs no transpose; diagonal 512-col
    chunks are trimmed to their causal width and masked only on the
    128-wide triangle block.
  - QK(j+1) is emitted before PV(j) so exp/mask latency hides under the
    next chunk's QK matmuls.
  - q/k/osb copies run on the otherwise-idle GpSimd (Pool) engine.
  - output projection for block Q is emitted after attention(Q+1, mo=0);
    aT is split per-mo so outproj's first half only waits on the mo=0
    normalization chain.
"""

import os
import sys

sys.path.insert(0, "/opt/trn_rl_repo")
os.environ.setdefault("MYCRO_LOCAL_CACHE", "1")

import numpy as np
import ml_dtypes

import concourse.bacc as bacc
import concourse.bass as bass
import concourse.mybir as mybir
import concourse.tile as tile
from concourse import bass_utils

# The agent image's antenv lacks axon_hooks, so bass_utils' trace path dies on
# import.  Register a shim module that lazily builds the ctypes NTFF hook.
if "antenv.axon_hooks" not in sys.modules:
    import types

    _shim = types.ModuleType("antenv.axon_hooks")
    _shim._HOOK = None

    def _set_hook(hook, _m=_shim):
        _m._HOOK = hook

    def _get_hook(_m=_shim):
        if _m._HOOK is None:
            try:
                from trn_agent_boot.trn_boot import _ntff_profile_via_ctypes

                _m._HOOK = _ntff_profile_via_ctypes("/opt/axon/libaxon_pjrt.so")
            except Exception:
                _m._HOOK = None
        return _m._HOOK

    _shim.set_axon_ntff_profile_hook = _set_hook
    _shim.get_axon_ntff_profile_hook = _get_hook
    sys.modules["antenv.axon_hooks"] = _shim

B, S, D, H = 2, 2048, 1024, 16
DK = 64                      # head dim
HC = 4                       # heads per core
GC = HC * DK                 # 256 cols per head-group
N_CORES = 8
SCALE = 1.0 / np.sqrt(DK)    # 0.125

F32 = mybir.dt.float32
BF16 = mybir.dt.bfloat16
NPBF16 = ml_dtypes.bfloat16

# which engine evacuates PSUM for q/k and osb copies (gpsimd frees ACT/DVE)
COPY_ENG = os.environ.get("BASS_COPY_ENG", "gpsimd")

TRACE = False
LAST_RESULTS = None


def build_bass():
    nc = bacc.Bacc("TRN2", target_bir_lowering=False, debug=False)

    xt_d = nc.dram_tensor("xt", [4, 128, 8 * 512], BF16, kind="ExternalInput")
    wq_d = nc.dram_tensor("wq", [128, 8, GC], BF16, kind="ExternalInput")
    wk_d = nc.dram_tensor("wk", [128, 8, GC], BF16, kind="ExternalInput")
    wv_d = nc.dram_tensor("wv", [128, 8, GC], BF16, kind="ExternalInput")
    wo_d = nc.dram_tensor("wo", [128, 2, D], BF16, kind="ExternalInput")
    tri_d = nc.dram_tensor("tri", [128, 128], BF16, kind="ExternalInput")
    out_d = nc.dram_tensor("out", [S, D], BF16, kind="ExternalOutput")

    EXP = mybir.ActivationFunctionType.Exp

    with tile.TileContext(nc) as tc:
        with (
            nc.allow_low_precision(reason="bf16 matmuls, fp32 psum accumulate"),
            tc.tile_pool(name="const", bufs=1) as const,
            tc.tile_pool(name="work", bufs=3) as work,
            tc.tile_pool(name="apool", bufs=4) as apool,
            tc.tile_pool(name="opool", bufs=3) as opool,
            tc.tile_pool(name="rpool", bufs=2) as rpool,
            tc.tile_pool(name="psmm", bufs=2, space="PSUM") as psmm,
            tc.tile_pool(name="psout", bufs=4, space="PSUM") as psout,
        ):
            copy_eng = getattr(nc, COPY_ENG)

            # ---- input DMAs, in consumption order ---------------------------
            wq = const.tile([128, 8, GC], BF16)
            nc.sync.dma_start(wq[:], wq_d[:])
            wk = const.tile([128, 8, GC], BF16)
            nc.sync.dma_start(wk[:], wk_d[:])
            xts = [const.tile([128, 8, 512], BF16, name=f"x{sb}")
                   for sb in range(4)]
            nc.sync.dma_start(xts[0].rearrange("p a s -> p (a s)"), xt_d[0])
            wv = const.tile([128, 8, GC], BF16)
            nc.sync.dma_start(wv[:], wv_d[:])
            tri = const.tile([128, 128], BF16)
            nc.sync.dma_start(tri[:], tri_d[:])
            for sb in (1, 2, 3):
                nc.sync.dma_start(xts[sb].rearrange("p a s -> p (a s)"), xt_d[sb])
            wo = const.tile([128, 2, D], BF16)
            nc.sync.dma_start(wo[:], wo_d[:])

            # v tiles: per j-chunk, per head 64 value cols + 64 ones cols (the
            # ones columns make the PV matmul emit the softmax denominator in
            # PSUM rows 64:128)
            vts = []
            for io in range(16):
                vt = const.tile([128, HC, 128], BF16, name=f"v{io}")
                nc.vector.memset(vt[:, :, 64:128], 1.0)
                vts.append(vt)

            qts = [[const.tile([128, 512], BF16, name=f"q{m}{s}")
                    for s in range(4)] for m in range(2)]
            kts = [[const.tile([128, 512], BF16, name=f"k{m}{s}")
                    for s in range(4)] for m in range(2)]

            def proj_qk(sb):
                for w_sb, dst in ((wq, qts), (wk, kts)):
                    for mo in range(2):
                        ps = psmm.tile([128, 2, 512], F32, tag="mm")
                        for ko in range(8):
                            nc.tensor.matmul(
                                ps[:, 0, :],
                                w_sb[:, ko, mo * 128:(mo + 1) * 128],
                                xts[sb][:, ko, :],
                                start=(ko == 0),
                                stop=(ko == 7),
                                skip_group_check=True,
                            )
                        copy_eng.tensor_copy(dst[mo][sb][:], ps[:, 0, :])

            def proj_v(io):
                sb, i2 = divmod(io, 4)
                ps = psmm.tile([128, 2, 512], F32, tag="mm")
                for ko in range(8):
                    nc.tensor.matmul(
                        ps[:, 0, 0:256],
                        xts[sb][:, ko, i2 * 128:(i2 + 1) * 128],
                        wv[:, ko, :],
                        start=(ko == 0),
                        stop=(ko == 7),
                        skip_group_check=True,
                    )
                nc.vector.tensor_copy(
                    vts[io][:, :, 0:64],
                    ps[:, 0, 0:256].rearrange("p (h e) -> p h e", e=64),
                )

            # ---- attention + output projection ------------------------------
            def attn(Q, mo, aTm):
                n_full = 4 * Q
                nch = n_full + 4
                out_ps = [psout.tile([128, 512], F32, tag="out",
                                     name=f"ops{Q}{mo}{_h}") for _h in range(2)]

                def qk(jc):
                    diag = jc >= n_full
                    o = jc - n_full if diag else 0
                    lo = o * 128 if diag else 0
                    sc = psmm.tile([128, 2, 512], F32, tag="mm")
                    for hp in range(2):
                        nc.tensor.matmul(
                            sc[:, hp, lo:512],
                            kts[mo][jc // 4][hp * 64:(hp + 1) * 64,
                                             (jc % 4) * 128:(jc % 4 + 1) * 128],
                            qts[mo][Q][hp * 64:(hp + 1) * 64, lo:512],
                            start=True,
                            stop=True,
                            skip_group_check=True,
                        )
                    ex = work.tile([128, 2, 512], BF16, tag="exp")
                    nc.scalar.activation(ex[:, :, lo:512], sc[:, :, lo:512],
                                         EXP, scale=SCALE)
                    if diag:
                        for hp in range(2):
                            nc.vector.tensor_mul(
                                ex[:, hp, lo:lo + 128],
                                ex[:, hp, lo:lo + 128],
                                tri[:],
                            )
                    return ex, lo

                def pv(jc, ex, lo):
                    for hp in range(2):
                        nc.tensor.matmul(
                            out_ps[hp][:, lo:512],
                            vts[jc][:, 2 * mo + hp, :],
                            ex[:, hp, lo:512],
                            start=(jc == 0),
                            stop=(jc == nch - 1),
                            skip_group_check=True,
                        )

                pend = qk(0)
                for jc in range(1, nch):
                    nxt = qk(jc)
                    pv(jc - 1, *pend)
                    pend = nxt
                pv(nch - 1, *pend)

                # normalization: rows 64:128 of out_ps hold the denominator
                # replicated 64x, so recip + mul are plain DVE ops.
                for hp in range(2):
                    den = rpool.tile([64, 512], F32, tag="den")
                    nc.vector.tensor_copy(den[:], out_ps[hp][64:128, :])
                    rdb = rpool.tile([64, 512], F32, tag="rd")
                    nc.vector.reciprocal_approx_fast(out=rdb[:], in_=den[:])
                    nc.vector.tensor_mul(
                        aTm[hp * 64:(hp + 1) * 64, :],
                        out_ps[hp][0:64, :],
                        rdb[:],
                    )

            def outproj(Q, aTq):
                for so in range(4):
                    po = psmm.tile([128, 2, 512], F32, tag="mm")
                    for co in range(2):
                        for nt in range(2):
                            nc.tensor.matmul(
                                po[:, nt, :],
                                aTq[co][:, so * 128:(so + 1) * 128],
                                wo[:, co, nt * 512:(nt + 1) * 512],
                                start=(co == 0),
                                stop=(co == 1),
                                skip_group_check=True,
                            )
                    osb = opool.tile([128, D], BF16, tag="osb")
                    copy_eng.tensor_copy(osb[:], po.rearrange("p a n -> p (a n)"))
                    nc.sync.dma_start(
                        out_d.rearrange("(a p) n -> p a n", p=128)[:, Q * 4 + so, :],
                        osb[:],
                    )

            aTs = []
            for Q in range(4):
                proj_qk(Q)
                for io in range(4 * Q, 4 * Q + 4):
                    proj_v(io)
                aTq = [apool.tile([128, 512], BF16, tag="aT", name=f"aT{Q}{m}")
                       for m in range(2)]
                aTs.append(aTq)
                attn(Q, 0, aTq[0])
                if Q >= 1:
                    outproj(Q - 1, aTs[Q - 1])
                attn(Q, 1, aTq[1])
            outproj(3, aTs[3])

    nc.compile()
    return nc


_NC = None


def _get_nc():
    global _NC
    if _NC is None:
        _NC = build_bass()
    return _NC


def _prep_core_inputs(x, Wq, Wk, Wv, Wo, c):
    b, g = divmod(c, 4)
    cols = slice(g * GC, (g + 1) * GC)
    xT = np.ascontiguousarray(x[b].T).astype(NPBF16)          # [1024, 2048]
    # xt[sb][p, ko*512+s] = xT[ko*128+p, sb*512+s]
    xt = np.ascontiguousarray(
        xT.reshape(8, 128, 4, 512).transpose(2, 1, 0, 3).reshape(4, 128, 8 * 512))

    def wtile(W):  # W.T[:, cols] -> [128, 8, 256]
        wt = np.ascontiguousarray(W.T[:, cols]).astype(NPBF16)
        return np.ascontiguousarray(wt.reshape(8, 128, GC).transpose(1, 0, 2))

    woT = np.ascontiguousarray(Wo[:, cols].T).astype(NPBF16)  # [256, 1024]
    wo = np.ascontiguousarray(woT.reshape(2, 128, D).transpose(1, 0, 2))
    tri = (np.arange(128)[:, None] <= np.arange(128)[None, :]).astype(NPBF16)
    return {
        "xt": xt,
        "wq": wtile(Wq),
        "wk": wtile(Wk),
        "wv": wtile(Wv),
        "wo": wo,
        "tri": tri,
    }


def kernel(in_features, Wq, Wk, Wv, Wo):
    global LAST_RESULTS
    nc = _get_nc()

    x = np.asarray(in_features, np.float32)
    Wq = np.asarray(Wq, np.float32)
    Wk = np.asarray(Wk, np.float32)
    Wv = np.asarray(Wv, np.float32)
    Wo = np.asarray(Wo, np.float32)

    in_maps = [_prep_core_inputs(x, Wq, Wk, Wv, Wo, c) for c in range(N_CORES)]

    res = bass_utils.run_bass_kernel_spmd(
        nc, in_maps, core_ids=list(range(N_CORES)), trace=TRACE,
    )
    LAST_RESULTS = res
    parts = [np.asarray(res.results[c]["out"], np.float32)
             for c in range(N_CORES)]
    out = np.stack([
        parts[4 * b] + parts[4 * b + 1] + parts[4 * b + 2] + parts[4 * b + 3]
        for b in range(B)
    ]).astype(np.float32)
    return out
